# revision 15
# baseline (speedup 1.0000x reference)
"""Trainium2 Bass kernel for nn_EnsembleNet (gnn_message_passing).

Computation (N=1024 nodes, T=4000, FH*FW=4096, D=H=128, C=10):
  xt = relu(waveforms @ W_time + b_time)            [N, D]
  xf = relu(spec.reshape(N,-1) @ W_freq + b_freq)   [N, D]
  At = normadj(xt), Af = normadj(xf)   (pairwise L1 -> 1/(d+eps), sym-norm)
  h  = relu(At @ (xt@W_gt) + Af @ (xf@W_gf) + b_g)  [N, H]
  out = h @ W_out + b_out                           [N, C]

Sharding: rows (nodes) split across 8 cores, 128 rows each; features are
all-gathered (bf16, 64KB/core).

Pairwise L1 as a GEMM (thermometer codes): with thresholds t_k=(k+.5)*Delta
and codes c_k(x) = (x > t_k) - 0.5 in {-.5, +.5}, the L1 distance between
the Delta-quantized values is EXACT:
  d1q[i,j] = (Delta/2) * (f_tot - sig_i.sig_j) = (Delta/2)*f_tot - 2*Delta*X'
where X' = C^T C is a plain matmul over f = D*K features. Quantization with
K=16, XMAX=4.5 adds ~2e-3 final l2 error (gate 2e-2). The local 128 node
codes are the stationary matmul weights, so the whole [128 x 1024] distance
block costs 16 weight loads + 32 512-col matmuls per adjacency on PE instead
of per-row DVE max + column matmuls. Diagonal: d1q_ii = 0 exactly, and
min(1/(d+eps),1) = 1/max(d+eps,1) is applied in log space via relu(ln).

Rows of a are complete on each core, so degrees come free from the Exp
activation's accum_out; only a 1KB dinv AllGather remains (the baseline's
196KB symmetric block-exchange collective is gone). dinv_i is folded into a
before the PE transposes, dinv_j into G after the gather.
"""

import os
import sys

import numpy as np
import ml_dtypes

# Self-contained path setup: the graded environment has the trn repo at one of
# these roots (PYTHONPATH normally provides it; make it explicit to be safe).
for _p in ("/opt/trn_rl_repo", "/root/.axon_site/_ro/trn_rl_repo"):
    if os.path.isdir(_p) and _p not in sys.path:
        sys.path.append(_p)

import concourse.bass as bass
import concourse.mybir as mybir
import concourse.tile as tile
from concourse import bacc
import concourse.hw_specs as _hw_specs
from concourse.bass_utils import run_bass_kernel_spmd

# All ACT functions this kernel uses (ln, exp, relu, copy) live in the
# "natural_log_exp_and_others" table set. The default per-function chooser
# picks the first set containing each function, reloading tables (~2.7us) on
# every ln<->exp alternation. Restrict the table map so one load serves all.
_orig_gat = _hw_specs.get_activation_tables


def _gat_combined(arch):
    t = _orig_gat(arch)
    return {name: (funcs if name == "natural_log_exp_and_others" else set())
            for name, funcs in t.items()}


bacc.get_activation_tables = _gat_combined

N = 1024
NCORES = 8
R = N // NCORES          # 128 rows per core
TPAD = 4096              # waveform length 4000, zero-padded to 4096
FHW = 4096               # 64*64 spectrogram
D = 128
H = 128
C = 10
EPS = 1e-5

K = 8                    # thermometer levels per feature dim
XMAX = 4.5               # quantization range for relu'd features
DELTA = XMAX / K
FTOT = D * K             # code feature dim per adjacency

BF16 = mybir.dt.bfloat16
F32 = mybir.dt.float32
AF = mybir.ActivationFunctionType
ALU = mybir.AluOpType

bf = ml_dtypes.bfloat16


def build_nc():
    nc = bacc.Bacc("TRN2", target_bir_lowering=False, debug=False,
                   num_devices=NCORES)

    # Per-core external inputs (host pre-transposes/casts/pads).
    # Big operands are partition-major [128, kin/128, 128] so each DMA reads
    # contiguous multi-KB runs per partition.
    NT = TPAD // 128
    wavesT = nc.dram_tensor("wavesT", [128, NT, R], BF16, kind="ExternalInput")
    specsT = nc.dram_tensor("specsT", [128, NT, R], BF16, kind="ExternalInput")
    wt = nc.dram_tensor("wt", [128, NT, D], BF16, kind="ExternalInput")
    wf = nc.dram_tensor("wf", [128, NT, D], BF16, kind="ExternalInput")
    wgt = nc.dram_tensor("wgt", [D, H], BF16, kind="ExternalInput")
    wgf = nc.dram_tensor("wgf", [D, H], BF16, kind="ExternalInput")
    wout = nc.dram_tensor("wout", [H, C], BF16, kind="ExternalInput")
    btime = nc.dram_tensor("btime", [1, D], BF16, kind="ExternalInput")
    bfreq = nc.dram_tensor("bfreq", [1, D], BF16, kind="ExternalInput")
    bg = nc.dram_tensor("bg", [1, H], BF16, kind="ExternalInput")
    bout = nc.dram_tensor("bout", [1, C], BF16, kind="ExternalInput")
    ident_in = nc.dram_tensor("ident", [128, 128], BF16, kind="ExternalInput")

    out_dram = nc.dram_tensor("out", [R, C], F32, kind="ExternalOutput")

    rg = [list(range(NCORES))]

    with tile.TileContext(nc) as tc:
        with (
            tc.tile_pool(name="const", bufs=1) as cpool,
            tc.tile_pool(name="stream", bufs=3) as spool,
            tc.tile_pool(name="tmp", bufs=2) as tpool,
            tc.tile_pool(name="psx", bufs=1, space="PSUM") as psbig,
            tc.tile_pool(name="pss", bufs=2, space="PSUM") as pssmall,
            tc.tile_pool(name="dram", bufs=1, space="DRAM") as dpool,
        ):
            # ---- constants ----
            ones_row = cpool.tile([1, 128], BF16, tag="ones_row")
            nc.gpsimd.memset(ones_row[:], 1.0)
            ident = cpool.tile([128, 128], BF16, tag="ident")
            nc.sync.dma_start(ident[:], ident_in[:])

            # Tiny dummy collective issued at t~0: the runtime's one-time
            # barrier + collectives init (~45us) then overlaps phase A
            # instead of serializing before the first real AllGather.
            ones8 = cpool.tile([1, 8], F32, tag="ones8")
            nc.gpsimd.memset(ones8[:], 1.0)
            ag0_in = dpool.tile([1, 8], F32, tag="ag0in")
            ag0_out = dpool.tile([NCORES, 1, 8], F32, tag="ag0out",
                                 addr_space="Shared")
            nc.sync.dma_start(ag0_in[:], ones8[:])
            nc.gpsimd.collective_compute(
                "AllGather", ALU.bypass, replica_groups=rg,
                ins=[ag0_in[:]], outs=[ag0_out[:]],
            )

            wg_sb = []
            for adj, wsrc in enumerate((wgt, wgf)):
                t = cpool.tile([D, H], BF16, tag=f"wg{adj}")
                nc.sync.dma_start(t[:], wsrc[:])
                wg_sb.append(t)
            wout_sb = cpool.tile([H, C], BF16, tag="wout")
            nc.sync.dma_start(wout_sb[:], wout[:])
            bias_sb = []
            for nm, src, width in (("btime", btime, D), ("bfreq", bfreq, D),
                                   ("bg", bg, H), ("bout", bout, C)):
                t = cpool.tile([1, width], BF16, tag=nm)
                nc.sync.dma_start(t[:], src[:])
                bias_sb.append(t)
            btime_sb, bfreq_sb, bg_sb, bout_sb = bias_sb

            # collective buffers (DRAM)
            ag1_in = dpool.tile([2, D, R], BF16, tag="ag1in")
            ag1_out = dpool.tile([NCORES, 2, D, R], BF16, tag="ag1out",
                                 addr_space="Shared")
            ag2_in = dpool.tile([R, 2], F32, tag="ag2in")
            ag2_out = dpool.tile([NCORES, R, 2], F32, tag="ag2out",
                                 addr_space="Shared")

            # Full gathered features [d, chunk, adj, node]; chunk m holds
            # global j-block (c+m)%8 so all compute is rank-invariant.
            xT_full = cpool.tile([128, NCORES, 2, 128], BF16, tag="xTfull")

            # ---- Phase A: input GEMMs -> local features into chunk 0 ----
            for adj, (xdram, wdram, bsb, kin) in enumerate(
                    ((wavesT, wt, btime_sb, TPAD), (specsT, wf, bfreq_sb, FHW))):
                psx = pssmall.tile([D, R], F32, tag="ps")
                nchunk = kin // 128
                wtile = spool.tile([128, nchunk, 128], BF16, bufs=1,
                                   tag=f"win{adj}", name=f"win{adj}")
                xtile = spool.tile([128, nchunk, 128], BF16, bufs=1,
                                   tag=f"xin{adj}", name=f"xin{adj}")
                for q in range(4):
                    sl = slice(q * nchunk // 4, (q + 1) * nchunk // 4)
                    nc.sync.dma_start(wtile[:, sl, :], wdram[:, sl, :])
                    nc.sync.dma_start(xtile[:, sl, :], xdram[:, sl, :])
                for b in range(nchunk):
                    nc.tensor.matmul(psx[:], wtile[:, b, :], xtile[:, b, :],
                                     start=(b == 0), stop=False)
                # bias row: psx[d, i] += b[d] * 1
                nc.tensor.matmul(psx[:], bsb[:], ones_row[:],
                                 start=False, stop=True)
                nc.scalar.activation(xT_full[:, 0, adj, :], psx[:], AF.Relu)
                nc.sync.dma_start(ag1_in[adj], xT_full[:, 0, adj, :])

            # ---- AllGather features; gather ROTATED per-core ----
            nc.gpsimd.collective_compute(
                "AllGather", ALU.bypass, replica_groups=rg,
                ins=[ag1_in[:]], outs=[ag1_out[:]],
            )
            pid = nc.partition_id()
            rot = [(pid + m) % NCORES for m in range(NCORES)]
            for m in range(1, NCORES):
                # src [1, 2, D, R] -> dst [d, 2, 128] at chunk m
                nc.sync.dma_start(
                    xT_full[:, m, :, :],
                    ag1_out[bass.ds(rot[m], 1)].rearrange("o a p f -> o p a f"))

            # ---- thermometer codes on DVE: c = (x > t_k) - 0.5 ----
            # F[adj][k] is [128(d), 1024(node)] bf16; half 0 = chunks 0..3.
            F_sb = cpool.tile([128, 2, K, N], BF16, tag="codes")
            HN = N // 2
            for adj in range(2):
                for half in range(2):
                    src = xT_full[:, 4 * half:4 * half + 4, adj, :]
                    for k in range(K):
                        tk = (k + 0.5) * DELTA
                        nc.vector.tensor_scalar(
                            F_sb[:, adj, k, half * HN:(half + 1) * HN],
                            src, tk, 0.5, op0=ALU.is_gt, op1=ALU.subtract)

            # ---- cross GEMM: X'[i_loc, j] = sum_f C[f,i]*C[f,j] ----
            # Local codes (chunk 0) are the stationary weights; one LDW per
            # (adj, k, half) feeds a 512-col matmul.
            ps_cross = [psbig.tile([128, N], F32, tag=f"cross{a}",
                                   name=f"cross{a}") for a in range(2)]
            for adj in range(2):
                for half in range(2):
                    for k in range(K):
                        nc.tensor.matmul(
                            ps_cross[adj][:, half * HN:(half + 1) * HN],
                            F_sb[:, adj, k, 0:128],
                            F_sb[:, adj, k, half * HN:(half + 1) * HN],
                            start=(k == 0), stop=(k == K - 1))

            # ---- post: a = 1/max(d1+eps, 1), deg_i = rowsum(a) ----
            # d1 = (DELTA/2)*FTOT - 2*DELTA*X'  (codes are +-0.5); the affine
            # folds into Ln's scale/bias, the clamp is relu in log space.
            c0 = (DELTA / 2.0) * FTOT + EPS
            c0_sb = cpool.tile([128, 1], F32, tag="c0")
            nc.gpsimd.memset(c0_sb[:], c0)
            a_sb = cpool.tile([128, 2, NCORES, 128], BF16, tag="a_sb")
            dinv_pack = tpool.tile([R, 2], F32, tag="dvpack", bufs=1)
            for adj in range(2):
                lnd = tpool.tile([128, N], F32, tag="lnd")
                nc.scalar.activation(lnd[:], ps_cross[adj][:], AF.Ln,
                                     scale=-2.0 * DELTA, bias=c0_sb[:])
                nc.scalar.activation(lnd[:], lnd[:], AF.Relu)
                deg = tpool.tile([R, 1], F32, tag=f"deg{adj}", bufs=1)
                nc.scalar.activation(a_sb[:, adj, :, :], lnd[:], AF.Exp,
                                     scale=-1.0, accum_out=deg[:])
                # dinv_i = rsqrt(deg) = exp(-0.5*ln(deg)); fold into a rows
                lr = tpool.tile([R, 1], F32, tag="lr")
                nc.scalar.activation(lr[:], deg[:], AF.Ln)
                dv = tpool.tile([R, 1], F32, tag=f"dv{adj}", bufs=1)
                nc.scalar.activation(dv[:], lr[:], AF.Exp, scale=-0.5)
                nc.vector.tensor_copy(dinv_pack[:, adj:adj + 1], dv[:])
                nc.vector.tensor_scalar(a_sb[:, adj, :, :], a_sb[:, adj, :, :],
                                        dv[:], None, op0=ALU.mult)
            nc.sync.dma_start(ag2_in[:], dinv_pack[:])

            nc.gpsimd.collective_compute(
                "AllGather", ALU.bypass, replica_groups=rg,
                ins=[ag2_in[:]], outs=[ag2_out[:]],
            )

            # ---- transpose a rows -> aT [j, i] chunks and G = X @ W_g
            # (PE work that overlaps AG2; transposes for adj emitted right
            # after its post-chain so PE never stalls on ACT).
            aT_sb = cpool.tile([128, 2, NCORES, 128], BF16, tag="aT_sb")
            G_sb = [cpool.tile([128, NCORES, H], BF16, tag=f"G{a}",
                               name=f"G{a}") for a in range(2)]
            for adj in range(2):
                for m in range(NCORES):
                    trp = pssmall.tile([128, 128], BF16, tag="ps",
                                       name=f"trp_{adj}_{m}")
                    nc.tensor.transpose(trp[:], a_sb[:, adj, m, :], ident[:])
                    nc.scalar.activation(aT_sb[:, adj, m, :], trp[:], AF.Copy)
                for m in range(NCORES):
                    psg = pssmall.tile([128, H], F32, tag="ps")
                    nc.tensor.matmul(psg[:], xT_full[:, m, adj, :],
                                     wg_sb[adj][:], start=True, stop=True)
                    nc.scalar.activation(G_sb[adj][:, m, :], psg[:], AF.Copy)

            # gather rotated dinvs: dinvs[:, m, adj] = dinv of global block;
            # spread descriptors across engine queues so the ~0.6us apiece
            # programming cost runs in parallel right after AG2.
            dinvs = cpool.tile([R, NCORES, 2], F32, tag="dinvs")
            gather_engines = [nc.sync, nc.scalar, nc.gpsimd]
            for m in range(NCORES):
                gather_engines[m % 3].dma_start(dinvs[:, m, :],
                                                ag2_out[bass.ds(rot[m], 1)])

            # ---- scale G by dinv_j, aggregate hT = sum G'^T(aT') + bgT ----
            # dinv_i already folded into a's rows pre-transpose, so both
            # adjacencies accumulate into one PSUM, transposed: hT[h, i].
            hT_ps = pssmall.tile([H, R], F32, tag="ps", name="hT_ps")
            nc.tensor.matmul(hT_ps[:], bg_sb[:], ones_row[:],
                             start=True, stop=False)
            for adj in range(2):
                for m in range(NCORES):
                    nc.vector.tensor_scalar(
                        G_sb[adj][:, m, :], G_sb[adj][:, m, :],
                        dinvs[:, m, adj:adj + 1], None, op0=ALU.mult)
                    nc.tensor.matmul(hT_ps[:], G_sb[adj][:, m, :],
                                     aT_sb[:, adj, m, :], start=False,
                                     stop=(adj == 1 and m == NCORES - 1))
            hT_bf = tpool.tile([H, R], BF16, tag="hT")
            nc.scalar.activation(hT_bf[:], hT_ps[:], AF.Relu)

            # out = h @ W_out + b_out (hT is already the lhsT layout)
            op = pssmall.tile([R, C], F32, tag="ps", name="op")
            nc.tensor.matmul(op[:], hT_bf[:], wout_sb[:], start=True, stop=False)
            nc.tensor.matmul(op[:], ones_row[:], bout_sb[:], start=False,
                             stop=True)
            out_sb = tpool.tile([R, C], F32, tag="osb")
            nc.vector.tensor_copy(out_sb[:], op[:])
            nc.sync.dma_start(out_dram[:], out_sb[:])

    nc.compile()
    return nc


_NC_CACHE = {}


def _get_nc():
    if "nc" not in _NC_CACHE:
        _NC_CACHE["nc"] = build_nc()
    return _NC_CACHE["nc"]


def _make_in_maps(inputs):
    waveforms = np.asarray(inputs["waveforms"], dtype=np.float32)
    spectrograms = np.asarray(inputs["spectrograms"], dtype=np.float32)
    W_time = np.asarray(inputs["W_time"], dtype=np.float32)
    W_freq = np.asarray(inputs["W_freq"], dtype=np.float32)
    W_gt = np.asarray(inputs["W_gt"], dtype=np.float32)
    W_gf = np.asarray(inputs["W_gf"], dtype=np.float32)
    W_out = np.asarray(inputs["W_out"], dtype=np.float32)
    b_time = np.asarray(inputs["b_time"], dtype=np.float32)
    b_freq = np.asarray(inputs["b_freq"], dtype=np.float32)
    b_g = np.asarray(inputs["b_g"], dtype=np.float32)
    b_out = np.asarray(inputs["b_out"], dtype=np.float32)

    T = waveforms.shape[1]

    def pmajor(arr_kN):
        # [KIN, 128] -> partition-major [128, KIN/128, 128]
        k = arr_kN.shape[0]
        return np.ascontiguousarray(
            arr_kN.reshape(k // 128, 128, -1).transpose(1, 0, 2))

    wt_pad = np.zeros((TPAD, D), dtype=bf)
    wt_pad[:T] = W_time.astype(bf)
    wf_b = W_freq.astype(bf)
    specs2 = spectrograms.reshape(N, FHW)

    common = dict(
        wt=pmajor(wt_pad),
        wf=pmajor(wf_b),
        wgt=np.ascontiguousarray(W_gt.astype(bf)),
        wgf=np.ascontiguousarray(W_gf.astype(bf)),
        wout=np.ascontiguousarray(W_out.astype(bf)),
        btime=np.ascontiguousarray(b_time.reshape(1, D).astype(bf)),
        bfreq=np.ascontiguousarray(b_freq.reshape(1, D).astype(bf)),
        bg=np.ascontiguousarray(b_g.reshape(1, H).astype(bf)),
        bout=np.ascontiguousarray(b_out.reshape(1, C).astype(bf)),
        ident=np.eye(128, dtype=bf),
    )
    in_maps = []
    for c in range(NCORES):
        rows = slice(c * R, (c + 1) * R)
        wT = np.zeros((TPAD, R), dtype=bf)
        wT[:T] = waveforms[rows].T.astype(bf)
        sT = specs2[rows].T.astype(bf)
        m = dict(common)
        m["wavesT"] = pmajor(wT)
        m["specsT"] = pmajor(sT)
        in_maps.append(m)
    return in_maps


def run(inputs, trace=False):
    nc = _get_nc()
    in_maps = _make_in_maps(inputs)
    res = run_bass_kernel_spmd(nc, in_maps, list(range(NCORES)), trace=trace)
    out = np.concatenate([res.results[c]["out"] for c in range(NCORES)], axis=0)
    return out.astype(np.float32), res


def kernel(**inputs):
    out, _ = run(inputs, trace=False)
    return out


# revision 21
# speedup vs baseline: 1.0848x; 1.0848x over previous
"""Trainium2 Bass kernel for nn_EnsembleNet (gnn_message_passing).

Computation (N=1024 nodes, T=4000, FH*FW=4096, D=H=128, C=10):
  xt = relu(waveforms @ W_time + b_time)            [N, D]
  xf = relu(spec.reshape(N,-1) @ W_freq + b_freq)   [N, D]
  At = normadj(xt), Af = normadj(xf)   (pairwise L1 -> 1/(d+eps), sym-norm)
  h  = relu(At @ (xt@W_gt) + Af @ (xf@W_gf) + b_g)  [N, H]
  out = h @ W_out + b_out                           [N, C]

Sharding: rows (nodes) split across 8 cores, 128 rows each; features are
all-gathered (bf16, 64KB/core).

Pairwise L1 as a GEMM (thermometer codes): with thresholds t_k=(k+.5)*Delta
and codes c_k(x) = (x > t_k) - 0.5 in {-.5, +.5}, the L1 distance between
the Delta-quantized values is EXACT:
  d1q[i,j] = (Delta/2) * (f_tot - sig_i.sig_j) = (Delta/2)*f_tot - 2*Delta*X'
where X' = C^T C is a plain matmul over f = D*K features. Quantization with
K=16, XMAX=4.5 adds ~2e-3 final l2 error (gate 2e-2). The local 128 node
codes are the stationary matmul weights, so the whole [128 x 1024] distance
block costs 16 weight loads + 32 512-col matmuls per adjacency on PE instead
of per-row DVE max + column matmuls. Diagonal: d1q_ii = 0 exactly, and
min(1/(d+eps),1) = 1/max(d+eps,1) is applied in log space via relu(ln).

Rows of a are complete on each core, so degrees come free from the Exp
activation's accum_out; only a 1KB dinv AllGather remains (the baseline's
196KB symmetric block-exchange collective is gone). dinv_i is folded into a
before the PE transposes, dinv_j into G after the gather.
"""

import os
import sys

import numpy as np
import ml_dtypes

# Self-contained path setup: the graded environment has the trn repo at one of
# these roots (PYTHONPATH normally provides it; make it explicit to be safe).
for _p in ("/opt/trn_rl_repo", "/root/.axon_site/_ro/trn_rl_repo"):
    if os.path.isdir(_p) and _p not in sys.path:
        sys.path.append(_p)

import concourse.bass as bass
import concourse.mybir as mybir
import concourse.tile as tile
from concourse import bacc
import concourse.hw_specs as _hw_specs
from concourse.bass_utils import run_bass_kernel_spmd

# All ACT functions this kernel uses (ln, exp, relu, copy) live in the
# "natural_log_exp_and_others" table set. The default per-function chooser
# picks the first set containing each function, reloading tables (~2.7us) on
# every ln<->exp alternation. Restrict the table map so one load serves all.
_orig_gat = _hw_specs.get_activation_tables


def _gat_combined(arch):
    t = _orig_gat(arch)
    return {name: (funcs if name == "natural_log_exp_and_others" else set())
            for name, funcs in t.items()}


bacc.get_activation_tables = _gat_combined

N = 1024
NCORES = 8
R = N // NCORES          # 128 rows per core
TPAD = 4096              # waveform length 4000, zero-padded to 4096
FHW = 4096               # 64*64 spectrogram
D = 128
H = 128
C = 10
EPS = 1e-5

K = 8                    # thermometer levels per feature dim
XMAX = 4.5               # quantization range for relu'd features
DELTA = XMAX / K
FTOT = D * K             # code feature dim per adjacency

BF16 = mybir.dt.bfloat16
F32 = mybir.dt.float32
AF = mybir.ActivationFunctionType
ALU = mybir.AluOpType

bf = ml_dtypes.bfloat16


def build_nc():
    nc = bacc.Bacc("TRN2", target_bir_lowering=False, debug=False,
                   num_devices=NCORES)

    # Per-core external inputs (host pre-transposes/casts/pads).
    # Big operands are partition-major [128, kin/128, 128] so each DMA reads
    # contiguous multi-KB runs per partition.
    NT = TPAD // 128
    wavesT = nc.dram_tensor("wavesT", [128, NT, R], BF16, kind="ExternalInput")
    specsT = nc.dram_tensor("specsT", [128, NT, R], BF16, kind="ExternalInput")
    wt = nc.dram_tensor("wt", [128, NT, D], BF16, kind="ExternalInput")
    wf = nc.dram_tensor("wf", [128, NT, D], BF16, kind="ExternalInput")
    wgt = nc.dram_tensor("wgt", [D, H], BF16, kind="ExternalInput")
    wgf = nc.dram_tensor("wgf", [D, H], BF16, kind="ExternalInput")
    wout = nc.dram_tensor("wout", [H, C], BF16, kind="ExternalInput")
    btime = nc.dram_tensor("btime", [1, D], BF16, kind="ExternalInput")
    bfreq = nc.dram_tensor("bfreq", [1, D], BF16, kind="ExternalInput")
    bg = nc.dram_tensor("bg", [1, H], BF16, kind="ExternalInput")
    bout = nc.dram_tensor("bout", [1, C], BF16, kind="ExternalInput")
    ident_in = nc.dram_tensor("ident", [128, 128], BF16, kind="ExternalInput")

    out_dram = nc.dram_tensor("out", [R, C], F32, kind="ExternalOutput")

    rg = [list(range(NCORES))]

    with tile.TileContext(nc) as tc:
        with (
            tc.tile_pool(name="const", bufs=1) as cpool,
            tc.tile_pool(name="stream", bufs=3) as spool,
            tc.tile_pool(name="tmp", bufs=2) as tpool,
            tc.tile_pool(name="psx", bufs=2, space="PSUM") as psbig,
            tc.tile_pool(name="pss", bufs=2, space="PSUM") as pssmall,
            tc.tile_pool(name="dram", bufs=1, space="DRAM") as dpool,
        ):
            # ---- constants ----
            ones_row = cpool.tile([1, 128], BF16, tag="ones_row")
            nc.gpsimd.memset(ones_row[:], 1.0)
            ident = cpool.tile([128, 128], BF16, tag="ident")
            nc.sync.dma_start(ident[:], ident_in[:])



            wg_sb = []
            for adj, wsrc in enumerate((wgt, wgf)):
                t = cpool.tile([D, H], BF16, tag=f"wg{adj}")
                nc.sync.dma_start(t[:], wsrc[:])
                wg_sb.append(t)
            wout_sb = cpool.tile([H, C], BF16, tag="wout")
            nc.sync.dma_start(wout_sb[:], wout[:])
            bias_sb = []
            for nm, src, width in (("btime", btime, D), ("bfreq", bfreq, D),
                                   ("bg", bg, H), ("bout", bout, C)):
                t = cpool.tile([1, width], BF16, tag=nm)
                nc.sync.dma_start(t[:], src[:])
                bias_sb.append(t)
            btime_sb, bfreq_sb, bg_sb, bout_sb = bias_sb

            # collective buffers (DRAM)
            ag1_in = dpool.tile([2, D, R], BF16, tag="ag1in")
            ag1_out = dpool.tile([NCORES, 2, D, R], BF16, tag="ag1out",
                                 addr_space="Shared")
            ag2_in = dpool.tile([R, 2], F32, tag="ag2in")
            ag2_out = dpool.tile([NCORES, R, 2], F32, tag="ag2out",
                                 addr_space="Shared")

            # Full gathered features [d, chunk, adj, node]; chunk m holds
            # global j-block (c+m)%8 so all compute is rank-invariant.
            xT_full = cpool.tile([128, NCORES, 2, 128], BF16, tag="xTfull")

            # ---- Phase A: input GEMMs -> local features into chunk 0 ----
            for adj, (xdram, wdram, bsb, kin) in enumerate(
                    ((wavesT, wt, btime_sb, TPAD), (specsT, wf, bfreq_sb, FHW))):
                psx = pssmall.tile([D, R], F32, tag="ps")
                nchunk = kin // 128
                wtile = spool.tile([128, nchunk, 128], BF16, bufs=1,
                                   tag=f"win{adj}", name=f"win{adj}")
                xtile = spool.tile([128, nchunk, 128], BF16, bufs=1,
                                   tag=f"xin{adj}", name=f"xin{adj}")
                for q in range(4):
                    sl = slice(q * nchunk // 4, (q + 1) * nchunk // 4)
                    nc.sync.dma_start(wtile[:, sl, :], wdram[:, sl, :])
                    nc.sync.dma_start(xtile[:, sl, :], xdram[:, sl, :])
                for b in range(nchunk):
                    nc.tensor.matmul(psx[:], wtile[:, b, :], xtile[:, b, :],
                                     start=(b == 0), stop=False)
                # bias row: psx[d, i] += b[d] * 1
                nc.tensor.matmul(psx[:], bsb[:], ones_row[:],
                                 start=False, stop=True)
                nc.scalar.activation(xT_full[:, 0, adj, :], psx[:], AF.Relu)
                nc.sync.dma_start(ag1_in[adj], xT_full[:, 0, adj, :])

            # ---- AllGather features; gather ROTATED per-core ----
            nc.gpsimd.collective_compute(
                "AllGather", ALU.bypass, replica_groups=rg,
                ins=[ag1_in[:]], outs=[ag1_out[:]],
            )
            pid = nc.partition_id()
            rot = [(pid + m) % NCORES for m in range(NCORES)]
            xg_engines = [nc.sync, nc.scalar, nc.gpsimd]
            for m in range(1, NCORES):
                # src [1, 2, D, R] -> dst [d, 2, 128] at chunk m; spread the
                # descriptor-programming cost across idle engine queues.
                xg_engines[(m - 1) % 3].dma_start(
                    xT_full[:, m, :, :],
                    ag1_out[bass.ds(rot[m], 1)].rearrange("o a p f -> o p a f"))

            # ---- thermometer codes on DVE: c = (x > t_k) - 0.5 ----
            # F[adj][k] is [128(d), 1024(node)] bf16; half 0 = chunks 0..3.
            F_sb = cpool.tile([128, 2, K, N], BF16, tag="codes")
            HN = N // 2
            for adj in range(2):
                for half in range(2):
                    src = xT_full[:, 4 * half:4 * half + 4, adj, :]
                    for k in range(K):
                        tk = (k + 0.5) * DELTA
                        nc.vector.tensor_scalar(
                            F_sb[:, adj, k, half * HN:(half + 1) * HN],
                            src, tk, 0.5, op0=ALU.is_gt, op1=ALU.subtract)

            # ---- cross GEMM: X'[i_loc, j] = sum_f C[f,i]*C[f,j] ----
            # Local codes (chunk 0) are the stationary weights; one LDW per
            # (adj, k, half) feeds a 512-col matmul.
            ps_cross = [psbig.tile([128, N], F32, tag="big",
                                   name=f"cross{a}") for a in range(2)]
            for adj in range(2):
                for half in range(2):
                    for k in range(K):
                        nc.tensor.matmul(
                            ps_cross[adj][:, half * HN:(half + 1) * HN],
                            F_sb[:, adj, k, 0:128],
                            F_sb[:, adj, k, half * HN:(half + 1) * HN],
                            start=(k == 0), stop=(k == K - 1))

            # ---- post: a = 1/max(d1+eps, 1), deg_i = rowsum(a) ----
            # d1 = (DELTA/2)*FTOT - 2*DELTA*X'  (codes are +-0.5); the affine
            # folds into Ln's scale/bias, the clamp is relu in log space.
            c0 = (DELTA / 2.0) * FTOT + EPS
            c0_sb = cpool.tile([128, 1], F32, tag="c0")
            nc.gpsimd.memset(c0_sb[:], c0)
            a_sb = cpool.tile([128, 2, NCORES, 128], BF16, tag="a_sb")
            dinv_pack = tpool.tile([R, 2], F32, tag="dvpack", bufs=1)
            for adj in range(2):
                lnd = tpool.tile([128, N], F32, tag="lnd")
                nc.scalar.activation(lnd[:], ps_cross[adj][:], AF.Ln,
                                     scale=-2.0 * DELTA, bias=c0_sb[:])
                # clamp in log space on DVE (ACT is the serial bottleneck)
                nc.vector.tensor_scalar(lnd[:], lnd[:], 0.0, None,
                                        op0=ALU.max)
                deg = tpool.tile([R, 1], F32, tag=f"deg{adj}", bufs=1)
                nc.scalar.activation(a_sb[:, adj, :, :], lnd[:], AF.Exp,
                                     scale=-1.0, accum_out=deg[:])
                # dinv_i = rsqrt(deg) = exp(-0.5*ln(deg)); fold into a rows
                lr = tpool.tile([R, 1], F32, tag="lr")
                nc.scalar.activation(lr[:], deg[:], AF.Ln)
                dv = tpool.tile([R, 1], F32, tag=f"dv{adj}", bufs=1)
                nc.scalar.activation(dv[:], lr[:], AF.Exp, scale=-0.5)
                nc.vector.tensor_copy(dinv_pack[:, adj:adj + 1], dv[:])
                nc.vector.tensor_scalar(a_sb[:, adj, :, :], a_sb[:, adj, :, :],
                                        dv[:], None, op0=ALU.mult)
            nc.sync.dma_start(ag2_in[:], dinv_pack[:])

            nc.gpsimd.collective_compute(
                "AllGather", ALU.bypass, replica_groups=rg,
                ins=[ag2_in[:]], outs=[ag2_out[:]],
            )

            # ---- transpose a rows -> aT [j, i] chunks and G = X @ W_g
            # (PE work that overlaps AG2; transposes for adj emitted right
            # after its post-chain so PE never stalls on ACT). The G GEMMs
            # land in the big PSUM pool (reusing the cross banks) and stay
            # there until the dinv scale copies them to SBUF.
            aT_sb = cpool.tile([128, 2, NCORES, 128], BF16, tag="aT_sb")
            G_sb = [cpool.tile([128, NCORES, H], BF16, tag=f"G{a}",
                               name=f"G{a}") for a in range(2)]
            gps = [psbig.tile([128, NCORES, H], F32, tag="big",
                              name=f"gps{a}") for a in range(2)]
            cp_engines = [nc.vector, nc.scalar]
            for adj in range(2):
                for m in range(NCORES):
                    trp = pssmall.tile([128, 128], BF16, tag="ps",
                                       name=f"trp_{adj}_{m}")
                    nc.tensor.transpose(trp[:], a_sb[:, adj, m, :], ident[:])
                    eng = cp_engines[m % 2]
                    if eng is nc.vector:
                        nc.vector.tensor_copy(aT_sb[:, adj, m, :], trp[:])
                    else:
                        nc.scalar.activation(aT_sb[:, adj, m, :], trp[:],
                                             AF.Copy)
                for m in range(NCORES):
                    nc.tensor.matmul(gps[adj][:, m, :], xT_full[:, m, adj, :],
                                     wg_sb[adj][:], start=True, stop=True)

            # gather rotated dinvs: dinvs[:, m, adj] = dinv of global block;
            # sync + gpsimd queues are idle here, scalar/vector are not.
            dinvs = cpool.tile([R, NCORES, 2], F32, tag="dinvs")
            gather_engines = [nc.sync, nc.gpsimd]
            for m in range(NCORES):
                gather_engines[m % 2].dma_start(dinvs[:, m, :],
                                               ag2_out[bass.ds(rot[m], 1)])

            # ---- scale G by dinv_j (PSUM -> SBUF, fused copy), aggregate
            # hT = sum G'^T(aT') + bgT. dinv_i already folded into a's rows
            # pre-transpose, so both adjacencies accumulate into one PSUM,
            # transposed: hT[h, i].
            hT_ps = pssmall.tile([H, R], F32, tag="ps", name="hT_ps")
            nc.tensor.matmul(hT_ps[:], bg_sb[:], ones_row[:],
                             start=True, stop=False)
            for adj in range(2):
                for m in range(NCORES):
                    nc.vector.tensor_scalar(
                        G_sb[adj][:, m, :], gps[adj][:, m, :],
                        dinvs[:, m, adj:adj + 1], None, op0=ALU.mult)
                    nc.tensor.matmul(hT_ps[:], G_sb[adj][:, m, :],
                                     aT_sb[:, adj, m, :], start=False,
                                     stop=(adj == 1 and m == NCORES - 1))
            hT_bf = tpool.tile([H, R], BF16, tag="hT")
            nc.scalar.activation(hT_bf[:], hT_ps[:], AF.Relu)

            # out = h @ W_out + b_out (hT is already the lhsT layout)
            op = pssmall.tile([R, C], F32, tag="ps", name="op")
            nc.tensor.matmul(op[:], hT_bf[:], wout_sb[:], start=True, stop=False)
            nc.tensor.matmul(op[:], ones_row[:], bout_sb[:], start=False,
                             stop=True)
            out_sb = tpool.tile([R, C], F32, tag="osb")
            nc.vector.tensor_copy(out_sb[:], op[:])
            nc.sync.dma_start(out_dram[:], out_sb[:])

    nc.compile()
    return nc


_NC_CACHE = {}


def _get_nc():
    if "nc" not in _NC_CACHE:
        _NC_CACHE["nc"] = build_nc()
    return _NC_CACHE["nc"]


def _make_in_maps(inputs):
    waveforms = np.asarray(inputs["waveforms"], dtype=np.float32)
    spectrograms = np.asarray(inputs["spectrograms"], dtype=np.float32)
    W_time = np.asarray(inputs["W_time"], dtype=np.float32)
    W_freq = np.asarray(inputs["W_freq"], dtype=np.float32)
    W_gt = np.asarray(inputs["W_gt"], dtype=np.float32)
    W_gf = np.asarray(inputs["W_gf"], dtype=np.float32)
    W_out = np.asarray(inputs["W_out"], dtype=np.float32)
    b_time = np.asarray(inputs["b_time"], dtype=np.float32)
    b_freq = np.asarray(inputs["b_freq"], dtype=np.float32)
    b_g = np.asarray(inputs["b_g"], dtype=np.float32)
    b_out = np.asarray(inputs["b_out"], dtype=np.float32)

    T = waveforms.shape[1]

    def pmajor(arr_kN):
        # [KIN, 128] -> partition-major [128, KIN/128, 128]
        k = arr_kN.shape[0]
        return np.ascontiguousarray(
            arr_kN.reshape(k // 128, 128, -1).transpose(1, 0, 2))

    wt_pad = np.zeros((TPAD, D), dtype=bf)
    wt_pad[:T] = W_time.astype(bf)
    wf_b = W_freq.astype(bf)
    specs2 = spectrograms.reshape(N, FHW)

    common = dict(
        wt=pmajor(wt_pad),
        wf=pmajor(wf_b),
        wgt=np.ascontiguousarray(W_gt.astype(bf)),
        wgf=np.ascontiguousarray(W_gf.astype(bf)),
        wout=np.ascontiguousarray(W_out.astype(bf)),
        btime=np.ascontiguousarray(b_time.reshape(1, D).astype(bf)),
        bfreq=np.ascontiguousarray(b_freq.reshape(1, D).astype(bf)),
        bg=np.ascontiguousarray(b_g.reshape(1, H).astype(bf)),
        bout=np.ascontiguousarray(b_out.reshape(1, C).astype(bf)),
        ident=np.eye(128, dtype=bf),
    )
    in_maps = []
    for c in range(NCORES):
        rows = slice(c * R, (c + 1) * R)
        wT = np.zeros((TPAD, R), dtype=bf)
        wT[:T] = waveforms[rows].T.astype(bf)
        sT = specs2[rows].T.astype(bf)
        m = dict(common)
        m["wavesT"] = pmajor(wT)
        m["specsT"] = pmajor(sT)
        in_maps.append(m)
    return in_maps


def run(inputs, trace=False):
    nc = _get_nc()
    in_maps = _make_in_maps(inputs)
    res = run_bass_kernel_spmd(nc, in_maps, list(range(NCORES)), trace=trace)
    out = np.concatenate([res.results[c]["out"] for c in range(NCORES)], axis=0)
    return out.astype(np.float32), res


def kernel(**inputs):
    out, _ = run(inputs, trace=False)
    return out


# revision 25
# speedup vs baseline: 1.1944x; 1.1011x over previous
"""Trainium2 Bass kernel for nn_EnsembleNet (gnn_message_passing).

Computation (N=1024 nodes, T=4000, FH*FW=4096, D=H=128, C=10):
  xt = relu(waveforms @ W_time + b_time)            [N, D]
  xf = relu(spec.reshape(N,-1) @ W_freq + b_freq)   [N, D]
  At = normadj(xt), Af = normadj(xf)   (pairwise L1 -> 1/(d+eps), sym-norm)
  h  = relu(At @ (xt@W_gt) + Af @ (xf@W_gf) + b_g)  [N, H]
  out = h @ W_out + b_out                           [N, C]

Sharding: rows (nodes) split across 8 cores, 128 rows each; features are
all-gathered (bf16, 64KB/core).

Pairwise L1 as a GEMM (thermometer codes): with thresholds t_k=(k+.5)*Delta
and codes c_k(x) = (x > t_k) - 0.5 in {-.5, +.5}, the L1 distance between
the Delta-quantized values is EXACT:
  d1q[i,j] = (Delta/2) * (f_tot - sig_i.sig_j) = (Delta/2)*f_tot - 2*Delta*X'
where X' = C^T C is a plain matmul over f = D*K features. Quantization with
K=16, XMAX=4.5 adds ~2e-3 final l2 error (gate 2e-2). The local 128 node
codes are the stationary matmul weights, so the whole [128 x 1024] distance
block costs 16 weight loads + 32 512-col matmuls per adjacency on PE instead
of per-row DVE max + column matmuls. Diagonal: d1q_ii = 0 exactly, and
min(1/(d+eps),1) = 1/max(d+eps,1) is applied in log space via relu(ln).

Rows of a are complete on each core, so degrees come free from the Exp
activation's accum_out; only a 1KB dinv AllGather remains (the baseline's
196KB symmetric block-exchange collective is gone). dinv_i is folded into a
before the PE transposes, dinv_j into G after the gather.
"""

import os
import sys

import numpy as np
import ml_dtypes

# Self-contained path setup: the graded environment has the trn repo at one of
# these roots (PYTHONPATH normally provides it; make it explicit to be safe).
for _p in ("/opt/trn_rl_repo", "/root/.axon_site/_ro/trn_rl_repo"):
    if os.path.isdir(_p) and _p not in sys.path:
        sys.path.append(_p)

import concourse.bass as bass
import concourse.mybir as mybir
import concourse.tile as tile
from concourse import bacc
import concourse.hw_specs as _hw_specs
from concourse.bass_utils import run_bass_kernel_spmd

# All ACT functions this kernel uses (ln, exp, relu, copy) live in the
# "natural_log_exp_and_others" table set. The default per-function chooser
# picks the first set containing each function, reloading tables (~2.7us) on
# every ln<->exp alternation. Restrict the table map so one load serves all.
_orig_gat = _hw_specs.get_activation_tables


def _gat_combined(arch):
    t = _orig_gat(arch)
    return {name: (funcs if name == "natural_log_exp_and_others" else set())
            for name, funcs in t.items()}


bacc.get_activation_tables = _gat_combined

N = 1024
NCORES = 8
R = N // NCORES          # 128 rows per core
TPAD = 4096              # waveform length 4000, zero-padded to 4096
FHW = 4096               # 64*64 spectrogram
D = 128
H = 128
C = 10
EPS = 1e-5

K = 8                    # thermometer levels per feature dim
XMAX = 4.5               # quantization range for relu'd features
DELTA = XMAX / K
FTOT = D * K             # code feature dim per adjacency

BF16 = mybir.dt.bfloat16
F32 = mybir.dt.float32
AF = mybir.ActivationFunctionType
ALU = mybir.AluOpType

bf = ml_dtypes.bfloat16


def build_nc():
    nc = bacc.Bacc("TRN2", target_bir_lowering=False, debug=False,
                   num_devices=NCORES)

    # Per-core external inputs (host pre-transposes/casts/pads).
    # Big operands are partition-major [128, kin/128, 128] so each DMA reads
    # contiguous multi-KB runs per partition.
    NT = TPAD // 128
    wavesT = nc.dram_tensor("wavesT", [128, NT, R], BF16, kind="ExternalInput")
    specsT = nc.dram_tensor("specsT", [128, NT, R], BF16, kind="ExternalInput")
    wt = nc.dram_tensor("wt", [128, NT, D], BF16, kind="ExternalInput")
    wf = nc.dram_tensor("wf", [128, NT, D], BF16, kind="ExternalInput")
    wgt = nc.dram_tensor("wgt", [D, H], BF16, kind="ExternalInput")
    wgf = nc.dram_tensor("wgf", [D, H], BF16, kind="ExternalInput")
    wout = nc.dram_tensor("wout", [H, C], BF16, kind="ExternalInput")
    btime = nc.dram_tensor("btime", [1, D], BF16, kind="ExternalInput")
    bfreq = nc.dram_tensor("bfreq", [1, D], BF16, kind="ExternalInput")
    bg = nc.dram_tensor("bg", [1, H], BF16, kind="ExternalInput")
    bout = nc.dram_tensor("bout", [1, C], BF16, kind="ExternalInput")
    ident_in = nc.dram_tensor("ident", [128, 128], BF16, kind="ExternalInput")

    out_dram = nc.dram_tensor("out", [R, C], F32, kind="ExternalOutput")

    rg = [list(range(NCORES))]

    with tile.TileContext(nc) as tc:
        with (
            tc.tile_pool(name="const", bufs=1) as cpool,
            tc.tile_pool(name="stream", bufs=3) as spool,
            tc.tile_pool(name="tmp", bufs=2) as tpool,
            tc.tile_pool(name="psx", bufs=2, space="PSUM") as psbig,
            tc.tile_pool(name="pss", bufs=2, space="PSUM") as pssmall,
            tc.tile_pool(name="psw", bufs=1, space="PSUM") as pswarm,
            tc.tile_pool(name="dram", bufs=1, space="DRAM") as dpool,
        ):
            # ---- constants ----
            ones_row = cpool.tile([1, 128], BF16, tag="ones_row")
            nc.gpsimd.memset(ones_row[:], 1.0)
            ident = cpool.tile([128, 128], BF16, tag="ident")
            nc.sync.dma_start(ident[:], ident_in[:])



            wg_sb = []
            for adj, wsrc in enumerate((wgt, wgf)):
                t = cpool.tile([D, H], BF16, tag=f"wg{adj}")
                nc.sync.dma_start(t[:], wsrc[:])
                wg_sb.append(t)
            wout_sb = cpool.tile([H, C], BF16, tag="wout")
            nc.sync.dma_start(wout_sb[:], wout[:])
            bias_sb = []
            for nm, src, width in (("btime", btime, D), ("bfreq", bfreq, D),
                                   ("bg", bg, H), ("bout", bout, C)):
                t = cpool.tile([1, width], BF16, tag=nm)
                nc.sync.dma_start(t[:], src[:])
                bias_sb.append(t)
            btime_sb, bfreq_sb, bg_sb, bout_sb = bias_sb

            # collective buffers (DRAM)
            ag1_in = dpool.tile([2, D, R], BF16, tag="ag1in")
            ag1_out = dpool.tile([NCORES, 2, D, R], BF16, tag="ag1out",
                                 addr_space="Shared")
            ag2_in = dpool.tile([R, 2], F32, tag="ag2in")
            ag2_out = dpool.tile([NCORES, R, 2], F32, tag="ag2out",
                                 addr_space="Shared")

            # Full gathered features [d, chunk, adj, node]; chunk m holds
            # global j-block (c+m)%8 so all compute is rank-invariant.
            xT_full = cpool.tile([128, NCORES, 2, 128], BF16, tag="xTfull")

            # ---- Phase A: input GEMMs -> local features into chunk 0 ----
            for adj, (xdram, wdram, bsb, kin) in enumerate(
                    ((wavesT, wt, btime_sb, TPAD), (specsT, wf, bfreq_sb, FHW))):
                psx = pssmall.tile([D, R], F32, tag="ps")
                nchunk = kin // 128
                wtile = spool.tile([128, nchunk, 128], BF16, bufs=1,
                                   tag=f"win{adj}", name=f"win{adj}")
                xtile = spool.tile([128, nchunk, 128], BF16, bufs=1,
                                   tag=f"xin{adj}", name=f"xin{adj}")
                for q in range(4):
                    sl = slice(q * nchunk // 4, (q + 1) * nchunk // 4)
                    nc.sync.dma_start(wtile[:, sl, :], wdram[:, sl, :])
                    nc.sync.dma_start(xtile[:, sl, :], xdram[:, sl, :])
                for b in range(nchunk):
                    nc.tensor.matmul(psx[:], wtile[:, b, :], xtile[:, b, :],
                                     start=(b == 0), stop=False)
                # bias row: psx[d, i] += b[d] * 1
                nc.tensor.matmul(psx[:], bsb[:], ones_row[:],
                                 start=False, stop=True)
                nc.scalar.activation(xT_full[:, 0, adj, :], psx[:], AF.Relu)
                nc.sync.dma_start(ag1_in[adj], xT_full[:, 0, adj, :])

            # ---- AllGather features; gather ROTATED per-core ----
            nc.gpsimd.collective_compute(
                "AllGather", ALU.bypass, replica_groups=rg,
                ins=[ag1_in[:]], outs=[ag1_out[:]],
            )
            pid = nc.partition_id()
            rot = [(pid + m) % NCORES for m in range(NCORES)]
            xg_engines = [nc.sync, nc.scalar, nc.gpsimd]
            for m in range(1, NCORES):
                # src [1, 2, D, R] -> dst [d, 2, 128] at chunk m; spread the
                # descriptor-programming cost across idle engine queues.
                xg_engines[(m - 1) % 3].dma_start(
                    xT_full[:, m, :, :],
                    ag1_out[bass.ds(rot[m], 1)].rearrange("o a p f -> o p a f"))

            # PE keep-warm: the tensor engine drops to a slow p-state after
            # idling (collective waits); dep-gated dummy transposes keep the
            # pipeline hot so the real GEMMs run at full clock.
            warm_ps = pswarm.tile([128, 128], BF16, tag="warm")

            def pe_warm(n, src):
                for _ in range(n):
                    nc.tensor.transpose(warm_ps[:], src, ident[:])

            # ramp back up across the AG1 gather while DVE builds codes
            pe_warm(14, xT_full[:, 1, 0, :])

            # ---- thermometer codes on DVE: c = (x > t_k) - 0.5 ----
            # F[adj][k] is [128(d), 1024(node)] bf16; half 0 = chunks 0..3.
            F_sb = cpool.tile([128, 2, K, N], BF16, tag="codes")
            HN = N // 2
            for adj in range(2):
                for half in range(2):
                    src = xT_full[:, 4 * half:4 * half + 4, adj, :]
                    for k in range(K):
                        tk = (k + 0.5) * DELTA
                        nc.vector.tensor_scalar(
                            F_sb[:, adj, k, half * HN:(half + 1) * HN],
                            src, tk, 0.5, op0=ALU.is_gt, op1=ALU.subtract)

            # ---- cross GEMM: X'[i_loc, j] = sum_f C[f,i]*C[f,j] ----
            # Local codes (chunk 0) are the stationary weights; one LDW per
            # (adj, k, half) feeds a 512-col matmul.
            ps_cross = [psbig.tile([128, N], F32, tag="big",
                                   name=f"cross{a}") for a in range(2)]
            for adj in range(2):
                for half in range(2):
                    for k in range(K):
                        nc.tensor.matmul(
                            ps_cross[adj][:, half * HN:(half + 1) * HN],
                            F_sb[:, adj, k, 0:128],
                            F_sb[:, adj, k, half * HN:(half + 1) * HN],
                            start=(k == 0), stop=(k == K - 1))

            # ---- post: a = 1/max(d1+eps, 1), deg_i = rowsum(a) ----
            # d1 = (DELTA/2)*FTOT - 2*DELTA*X'  (codes are +-0.5); the affine
            # folds into Ln's scale/bias, the clamp is relu in log space.
            c0 = (DELTA / 2.0) * FTOT + EPS
            c0_sb = cpool.tile([128, 1], F32, tag="c0")
            nc.gpsimd.memset(c0_sb[:], c0)
            a_sb = cpool.tile([128, 2, NCORES, 128], BF16, tag="a_sb")
            dinv_pack = tpool.tile([R, 2], F32, tag="dvpack", bufs=1)
            for adj in range(2):
                lnd = tpool.tile([128, N], F32, tag="lnd")
                nc.scalar.activation(lnd[:], ps_cross[adj][:], AF.Ln,
                                     scale=-2.0 * DELTA, bias=c0_sb[:])
                # clamp in log space: relu(ln d) <=> max(d, 1). On ACT: the
                # DVE queue is still draining code ops at this point.
                nc.scalar.activation(lnd[:], lnd[:], AF.Relu)
                deg = tpool.tile([R, 1], F32, tag=f"deg{adj}", bufs=1)
                nc.scalar.activation(a_sb[:, adj, :, :], lnd[:], AF.Exp,
                                     scale=-1.0, accum_out=deg[:])
                # dinv_i = rsqrt(deg) = exp(-0.5*ln(deg)); fold into a rows
                lr = tpool.tile([R, 1], F32, tag="lr")
                nc.scalar.activation(lr[:], deg[:], AF.Ln)
                dv = tpool.tile([R, 1], F32, tag=f"dv{adj}", bufs=1)
                nc.scalar.activation(dv[:], lr[:], AF.Exp, scale=-0.5)
                nc.vector.tensor_copy(dinv_pack[:, adj:adj + 1], dv[:])
                nc.vector.tensor_scalar(a_sb[:, adj, :, :], a_sb[:, adj, :, :],
                                        dv[:], None, op0=ALU.mult)
            nc.sync.dma_start(ag2_in[:], dinv_pack[:])

            nc.gpsimd.collective_compute(
                "AllGather", ALU.bypass, replica_groups=rg,
                ins=[ag2_in[:]], outs=[ag2_out[:]],
            )

            # ---- transpose a rows -> aT [j, i] chunks and G = X @ W_g
            # (PE work that overlaps AG2; transposes for adj emitted right
            # after its post-chain so PE never stalls on ACT). The G GEMMs
            # land in the big PSUM pool (reusing the cross banks) and stay
            # there until the dinv scale copies them to SBUF.
            aT_sb = cpool.tile([128, 2, NCORES, 128], BF16, tag="aT_sb")
            G_sb = [cpool.tile([128, NCORES, H], BF16, tag=f"G{a}",
                               name=f"G{a}") for a in range(2)]
            gps = [psbig.tile([128, NCORES, H], F32, tag="big",
                              name=f"gps{a}") for a in range(2)]
            cp_engines = [nc.vector, nc.scalar]
            for adj in range(2):
                for m in range(NCORES):
                    trp = pssmall.tile([128, 128], BF16, tag="ps",
                                       name=f"trp_{adj}_{m}")
                    nc.tensor.transpose(trp[:], a_sb[:, adj, m, :], ident[:])
                    eng = cp_engines[m % 2]
                    if eng is nc.vector:
                        nc.vector.tensor_copy(aT_sb[:, adj, m, :], trp[:])
                    else:
                        nc.scalar.activation(aT_sb[:, adj, m, :], trp[:],
                                             AF.Copy)
                for m in range(NCORES):
                    nc.tensor.matmul(gps[adj][:, m, :], xT_full[:, m, adj, :],
                                     wg_sb[adj][:], start=True, stop=True)

            # keep PE hot across the AG2 wait so the aggregation GEMMs run
            # at full clock (dep-free: these run back-to-back after G)
            pe_warm(36, a_sb[:, 0, 0, :])

            # gather rotated dinvs: dinvs[:, m, adj] = dinv of global block;
            # sync + gpsimd queues are idle here, scalar/vector are not.
            dinvs = cpool.tile([R, NCORES, 2], F32, tag="dinvs")
            gather_engines = [nc.sync, nc.gpsimd]
            for m in range(NCORES):
                gather_engines[m % 2].dma_start(dinvs[:, m, :],
                                               ag2_out[bass.ds(rot[m], 1)])

            # ---- scale G by dinv_j (PSUM -> SBUF, fused copy), aggregate
            # hT = sum G'^T(aT') + bgT. dinv_i already folded into a's rows
            # pre-transpose, so both adjacencies accumulate into one PSUM,
            # transposed: hT[h, i].
            hT_ps = pssmall.tile([H, R], F32, tag="ps", name="hT_ps")
            nc.tensor.matmul(hT_ps[:], bg_sb[:], ones_row[:],
                             start=True, stop=False)
            for adj in range(2):
                for m in range(NCORES):
                    nc.vector.tensor_scalar(
                        G_sb[adj][:, m, :], gps[adj][:, m, :],
                        dinvs[:, m, adj:adj + 1], None, op0=ALU.mult)
                    nc.tensor.matmul(hT_ps[:], G_sb[adj][:, m, :],
                                     aT_sb[:, adj, m, :], start=False,
                                     stop=(adj == 1 and m == NCORES - 1))
            hT_bf = tpool.tile([H, R], BF16, tag="hT")
            nc.scalar.activation(hT_bf[:], hT_ps[:], AF.Relu)

            # out = h @ W_out + b_out (hT is already the lhsT layout)
            op = pssmall.tile([R, C], F32, tag="ps", name="op")
            nc.tensor.matmul(op[:], hT_bf[:], wout_sb[:], start=True, stop=False)
            nc.tensor.matmul(op[:], ones_row[:], bout_sb[:], start=False,
                             stop=True)
            out_sb = tpool.tile([R, C], F32, tag="osb")
            nc.vector.tensor_copy(out_sb[:], op[:])
            nc.sync.dma_start(out_dram[:], out_sb[:])

    nc.compile()
    return nc


_NC_CACHE = {}


def _get_nc():
    if "nc" not in _NC_CACHE:
        _NC_CACHE["nc"] = build_nc()
    return _NC_CACHE["nc"]


def _make_in_maps(inputs):
    waveforms = np.asarray(inputs["waveforms"], dtype=np.float32)
    spectrograms = np.asarray(inputs["spectrograms"], dtype=np.float32)
    W_time = np.asarray(inputs["W_time"], dtype=np.float32)
    W_freq = np.asarray(inputs["W_freq"], dtype=np.float32)
    W_gt = np.asarray(inputs["W_gt"], dtype=np.float32)
    W_gf = np.asarray(inputs["W_gf"], dtype=np.float32)
    W_out = np.asarray(inputs["W_out"], dtype=np.float32)
    b_time = np.asarray(inputs["b_time"], dtype=np.float32)
    b_freq = np.asarray(inputs["b_freq"], dtype=np.float32)
    b_g = np.asarray(inputs["b_g"], dtype=np.float32)
    b_out = np.asarray(inputs["b_out"], dtype=np.float32)

    T = waveforms.shape[1]

    def pmajor(arr_kN):
        # [KIN, 128] -> partition-major [128, KIN/128, 128]
        k = arr_kN.shape[0]
        return np.ascontiguousarray(
            arr_kN.reshape(k // 128, 128, -1).transpose(1, 0, 2))

    wt_pad = np.zeros((TPAD, D), dtype=bf)
    wt_pad[:T] = W_time.astype(bf)
    wf_b = W_freq.astype(bf)
    specs2 = spectrograms.reshape(N, FHW)

    common = dict(
        wt=pmajor(wt_pad),
        wf=pmajor(wf_b),
        wgt=np.ascontiguousarray(W_gt.astype(bf)),
        wgf=np.ascontiguousarray(W_gf.astype(bf)),
        wout=np.ascontiguousarray(W_out.astype(bf)),
        btime=np.ascontiguousarray(b_time.reshape(1, D).astype(bf)),
        bfreq=np.ascontiguousarray(b_freq.reshape(1, D).astype(bf)),
        bg=np.ascontiguousarray(b_g.reshape(1, H).astype(bf)),
        bout=np.ascontiguousarray(b_out.reshape(1, C).astype(bf)),
        ident=np.eye(128, dtype=bf),
    )
    in_maps = []
    for c in range(NCORES):
        rows = slice(c * R, (c + 1) * R)
        wT = np.zeros((TPAD, R), dtype=bf)
        wT[:T] = waveforms[rows].T.astype(bf)
        sT = specs2[rows].T.astype(bf)
        m = dict(common)
        m["wavesT"] = pmajor(wT)
        m["specsT"] = pmajor(sT)
        in_maps.append(m)
    return in_maps


def run(inputs, trace=False):
    nc = _get_nc()
    in_maps = _make_in_maps(inputs)
    res = run_bass_kernel_spmd(nc, in_maps, list(range(NCORES)), trace=trace)
    out = np.concatenate([res.results[c]["out"] for c in range(NCORES)], axis=0)
    return out.astype(np.float32), res


def kernel(**inputs):
    out, _ = run(inputs, trace=False)
    return out


# revision 30
# speedup vs baseline: 1.2592x; 1.0542x over previous
"""Trainium2 Bass kernel for nn_EnsembleNet (gnn_message_passing).

Computation (N=1024 nodes, T=4000, FH*FW=4096, D=H=128, C=10):
  xt = relu(waveforms @ W_time + b_time)            [N, D]
  xf = relu(spec.reshape(N,-1) @ W_freq + b_freq)   [N, D]
  At = normadj(xt), Af = normadj(xf)   (pairwise L1 -> 1/(d+eps), sym-norm)
  h  = relu(At @ (xt@W_gt) + Af @ (xf@W_gf) + b_g)  [N, H]
  out = h @ W_out + b_out                           [N, C]

Sharding: rows (nodes) split across 8 cores, 128 rows each; features are
all-gathered (bf16, 64KB/core).

Pairwise L1 as a GEMM (thermometer codes): with thresholds t_k=(k+.5)*Delta
and codes c_k(x) = (x > t_k) - 0.5 in {-.5, +.5}, the L1 distance between
the Delta-quantized values is EXACT:
  d1q[i,j] = (Delta/2) * (f_tot - sig_i.sig_j) = (Delta/2)*f_tot - 2*Delta*X'
where X' = C^T C is a plain matmul over f = D*K features. Quantization with
K=16, XMAX=4.5 adds ~2e-3 final l2 error (gate 2e-2). The local 128 node
codes are the stationary matmul weights, so the whole [128 x 1024] distance
block costs 16 weight loads + 32 512-col matmuls per adjacency on PE instead
of per-row DVE max + column matmuls. Diagonal: d1q_ii = 0 exactly, and
min(1/(d+eps),1) = 1/max(d+eps,1) is applied in log space via relu(ln).

Rows of a are complete on each core, so degrees come free from the Exp
activation's accum_out; only a 1KB dinv AllGather remains (the baseline's
196KB symmetric block-exchange collective is gone). dinv_i is folded into a
before the PE transposes, dinv_j into G after the gather.
"""

import os
import sys

import numpy as np
import ml_dtypes

# Self-contained path setup: the graded environment has the trn repo at one of
# these roots (PYTHONPATH normally provides it; make it explicit to be safe).
for _p in ("/opt/trn_rl_repo", "/root/.axon_site/_ro/trn_rl_repo"):
    if os.path.isdir(_p) and _p not in sys.path:
        sys.path.append(_p)

import concourse.bass as bass
import concourse.mybir as mybir
import concourse.tile as tile
from concourse import bacc
import concourse.hw_specs as _hw_specs
from concourse.bass_utils import run_bass_kernel_spmd

# All ACT functions this kernel uses (ln, exp, relu, copy) live in the
# "natural_log_exp_and_others" table set. The default per-function chooser
# picks the first set containing each function, reloading tables (~2.7us) on
# every ln<->exp alternation. Restrict the table map so one load serves all.
_orig_gat = _hw_specs.get_activation_tables


def _gat_combined(arch):
    t = _orig_gat(arch)
    return {name: (funcs if name == "natural_log_exp_and_others" else set())
            for name, funcs in t.items()}


bacc.get_activation_tables = _gat_combined

N = 1024
NCORES = 8
R = N // NCORES          # 128 rows per core
TPAD = 4096              # waveform length 4000, zero-padded to 4096
FHW = 4096               # 64*64 spectrogram
D = 128
H = 128
C = 10
EPS = 1e-5

K = 8                    # thermometer levels per feature dim
XMAX = 4.5               # quantization range for relu'd features
DELTA = XMAX / K
FTOT = D * K             # code feature dim per adjacency

BF16 = mybir.dt.bfloat16
F32 = mybir.dt.float32
AF = mybir.ActivationFunctionType
ALU = mybir.AluOpType

bf = ml_dtypes.bfloat16


def build_nc():
    nc = bacc.Bacc("TRN2", target_bir_lowering=False, debug=False,
                   num_devices=NCORES)

    # Per-core external inputs (host pre-transposes/casts/pads).
    # Big operands are partition-major [128, kin/128, 128] so each DMA reads
    # contiguous multi-KB runs per partition.
    NT = TPAD // 128
    wavesT = nc.dram_tensor("wavesT", [128, NT, R], BF16, kind="ExternalInput")
    specsT = nc.dram_tensor("specsT", [128, NT, R], BF16, kind="ExternalInput")
    wt = nc.dram_tensor("wt", [128, NT, D], BF16, kind="ExternalInput")
    wf = nc.dram_tensor("wf", [128, NT, D], BF16, kind="ExternalInput")
    wgt = nc.dram_tensor("wgt", [D, H], BF16, kind="ExternalInput")
    wgf = nc.dram_tensor("wgf", [D, H], BF16, kind="ExternalInput")
    wout = nc.dram_tensor("wout", [H, C], BF16, kind="ExternalInput")
    btime = nc.dram_tensor("btime", [1, D], BF16, kind="ExternalInput")
    bfreq = nc.dram_tensor("bfreq", [1, D], BF16, kind="ExternalInput")
    bg = nc.dram_tensor("bg", [1, H], BF16, kind="ExternalInput")
    bout = nc.dram_tensor("bout", [1, C], BF16, kind="ExternalInput")
    ident_in = nc.dram_tensor("ident", [128, 128], BF16, kind="ExternalInput")

    out_dram = nc.dram_tensor("out", [R, C], F32, kind="ExternalOutput")

    rg = [list(range(NCORES))]

    with tile.TileContext(nc) as tc:
        with (
            tc.tile_pool(name="const", bufs=1) as cpool,
            tc.tile_pool(name="stream", bufs=3) as spool,
            tc.tile_pool(name="tmp", bufs=2) as tpool,
            tc.tile_pool(name="psx", bufs=2, space="PSUM") as psbig,
            tc.tile_pool(name="pss", bufs=2, space="PSUM") as pssmall,
            tc.tile_pool(name="psw", bufs=1, space="PSUM") as pswarm,
            tc.tile_pool(name="dram", bufs=1, space="DRAM") as dpool,
        ):
            # ---- constants ----
            ones_row = cpool.tile([1, 128], BF16, tag="ones_row")
            nc.gpsimd.memset(ones_row[:], 1.0)
            ident = cpool.tile([128, 128], BF16, tag="ident")
            nc.sync.dma_start(ident[:], ident_in[:])



            wg_sb = []
            for adj, wsrc in enumerate((wgt, wgf)):
                t = cpool.tile([D, H], BF16, tag=f"wg{adj}")
                nc.sync.dma_start(t[:], wsrc[:])
                wg_sb.append(t)
            wout_sb = cpool.tile([H, C], BF16, tag="wout")
            nc.sync.dma_start(wout_sb[:], wout[:])
            bias_sb = []
            for nm, src, width in (("btime", btime, D), ("bfreq", bfreq, D),
                                   ("bg", bg, H), ("bout", bout, C)):
                t = cpool.tile([1, width], BF16, tag=nm)
                nc.sync.dma_start(t[:], src[:])
                bias_sb.append(t)
            btime_sb, bfreq_sb, bg_sb, bout_sb = bias_sb

            # collective buffers (DRAM)
            ag1_in = dpool.tile([2, D, R], BF16, tag="ag1in")
            ag1_out = dpool.tile([NCORES, 2, D, R], BF16, tag="ag1out",
                                 addr_space="Shared")
            ag2_in = dpool.tile([2, R, H], BF16, tag="ag2in")
            ag2_out = dpool.tile([NCORES, 2, R, H], BF16, tag="ag2out",
                                 addr_space="Shared")

            # Full gathered features [d, chunk, adj, node]; chunk m holds
            # global j-block (c+m)%8 so all compute is rank-invariant.
            xT_full = cpool.tile([128, NCORES, 2, 128], BF16, tag="xTfull")

            # ---- Phase A: input GEMMs -> local features into chunk 0 ----
            for adj, (xdram, wdram, bsb, kin) in enumerate(
                    ((wavesT, wt, btime_sb, TPAD), (specsT, wf, bfreq_sb, FHW))):
                psx = pssmall.tile([D, R], F32, tag="ps")
                nchunk = kin // 128
                wtile = spool.tile([128, nchunk, 128], BF16, bufs=1,
                                   tag=f"win{adj}", name=f"win{adj}")
                xtile = spool.tile([128, nchunk, 128], BF16, bufs=1,
                                   tag=f"xin{adj}", name=f"xin{adj}")
                for q in range(4):
                    sl = slice(q * nchunk // 4, (q + 1) * nchunk // 4)
                    nc.sync.dma_start(wtile[:, sl, :], wdram[:, sl, :])
                    nc.sync.dma_start(xtile[:, sl, :], xdram[:, sl, :])
                for b in range(nchunk):
                    nc.tensor.matmul(psx[:], wtile[:, b, :], xtile[:, b, :],
                                     start=(b == 0), stop=False)
                # bias row: psx[d, i] += b[d] * 1
                nc.tensor.matmul(psx[:], bsb[:], ones_row[:],
                                 start=False, stop=True)
                nc.scalar.activation(xT_full[:, 0, adj, :], psx[:], AF.Relu)
                nc.sync.dma_start(ag1_in[adj], xT_full[:, 0, adj, :])

            # G for the local rows only: G_own[j, h] = xt_own^T @ Wg. Held in
            # PSUM until deg/dinv arrives, then dinv-scaled, shipped, and
            # all-gathered -- remote cores never compute or scale G.
            psg_own = pswarm.tile([128, 2, H], F32, tag="psg")
            for adj in range(2):
                nc.tensor.matmul(psg_own[:, adj, :], xT_full[:, 0, adj, :],
                                 wg_sb[adj][:], start=True, stop=True)

            # ---- AllGather features; gather ROTATED per-core ----
            nc.gpsimd.collective_compute(
                "AllGather", ALU.bypass, replica_groups=rg,
                ins=[ag1_in[:]], outs=[ag1_out[:]],
            )
            pid = nc.partition_id()
            rot = [(pid + m) % NCORES for m in range(NCORES)]
            xg_engines = [nc.sync, nc.scalar, nc.gpsimd]
            for m in range(1, NCORES):
                # src [1, 2, D, R] -> dst [d, 2, 128] at chunk m; spread the
                # descriptor-programming cost across idle engine queues.
                xg_engines[(m - 1) % 3].dma_start(
                    xT_full[:, m, :, :],
                    ag1_out[bass.ds(rot[m], 1)].rearrange("o a p f -> o p a f"))

            # PE keep-warm: the tensor engine drops to a slow p-state after
            # idling (collective waits); dep-gated dummy transposes keep the
            # pipeline hot so the real GEMMs run at full clock.
            warm_ps = pswarm.tile([128, 128], BF16, tag="warm")

            def pe_warm(n, src):
                for _ in range(n):
                    nc.tensor.transpose(warm_ps[:], src, ident[:])

            # ramp back up across the AG1 gather while DVE builds codes
            pe_warm(14, xT_full[:, 1, 0, :])

            # ---- thermometer codes on DVE: c = (x > t_k) - 0.5 ----
            # F[adj][k] is [128(d), 1024(node)] bf16; half 0 = chunks 0..3.
            F_sb = cpool.tile([128, 2, K, N], BF16, tag="codes")
            HN = N // 2
            for adj in range(2):
                for half in range(2):
                    src = xT_full[:, 4 * half:4 * half + 4, adj, :]
                    for k in range(K):
                        tk = (k + 0.5) * DELTA
                        nc.vector.tensor_scalar(
                            F_sb[:, adj, k, half * HN:(half + 1) * HN],
                            src, tk, 0.5, op0=ALU.is_gt, op1=ALU.subtract)

            # ---- cross GEMM: X'[i_loc, j] = sum_f C[f,i]*C[f,j] ----
            # Local codes (chunk 0) are the stationary weights; one LDW per
            # (adj, k, half) feeds a 512-col matmul.
            ps_cross = [psbig.tile([128, N], F32, tag="big",
                                   name=f"cross{a}") for a in range(2)]
            for adj in range(2):
                for half in range(2):
                    for k in range(K):
                        nc.tensor.matmul(
                            ps_cross[adj][:, half * HN:(half + 1) * HN],
                            F_sb[:, adj, k, 0:128],
                            F_sb[:, adj, k, half * HN:(half + 1) * HN],
                            start=(k == 0), stop=(k == K - 1))

            # ---- post: a = 1/max(d1+eps, 1), deg_i = rowsum(a) ----
            # d1 = (DELTA/2)*FTOT - 2*DELTA*X'  (codes are +-0.5); the affine
            # folds into Ln's scale/bias, the clamp is relu in log space.
            c0 = (DELTA / 2.0) * FTOT + EPS
            c0_sb = cpool.tile([128, 1], F32, tag="c0")
            nc.gpsimd.memset(c0_sb[:], c0)
            a_sb = cpool.tile([128, 2, NCORES, 128], BF16, tag="a_sb")
            # gathered dinv-scaled G chunks [j, chunk, adj, h]; chunk 0 is
            # written locally by the dinv scale below.
            G_full = cpool.tile([128, NCORES, 2, H], BF16, tag="G_full")
            for adj in range(2):
                lnd = tpool.tile([128, N], F32, tag="lnd")
                nc.scalar.activation(lnd[:], ps_cross[adj][:], AF.Ln,
                                     scale=-2.0 * DELTA, bias=c0_sb[:])
                # clamp in log space: relu(ln d) <=> max(d, 1). On ACT: the
                # DVE queue is still draining code ops at this point.
                nc.scalar.activation(lnd[:], lnd[:], AF.Relu)
                deg = tpool.tile([R, 1], F32, tag=f"deg{adj}", bufs=1)
                nc.scalar.activation(a_sb[:, adj, :, :], lnd[:], AF.Exp,
                                     scale=-1.0, accum_out=deg[:])
                # dinv_i = rsqrt(deg) = exp(-0.5*ln(deg)); fold into a rows
                lr = tpool.tile([R, 1], F32, tag="lr")
                nc.scalar.activation(lr[:], deg[:], AF.Ln)
                dv = tpool.tile([R, 1], F32, tag=f"dv{adj}", bufs=1)
                nc.scalar.activation(dv[:], lr[:], AF.Exp, scale=-0.5)
                nc.vector.tensor_scalar(a_sb[:, adj, :, :], a_sb[:, adj, :, :],
                                        dv[:], None, op0=ALU.mult)
                nc.vector.tensor_scalar(G_full[:, 0, adj, :],
                                        psg_own[:, adj, :], dv[:], None,
                                        op0=ALU.mult)
            nc.sync.dma_start(ag2_in[:].rearrange("a p f -> p a f"),
                              G_full[:, 0, :, :])

            nc.gpsimd.collective_compute(
                "AllGather", ALU.bypass, replica_groups=rg,
                ins=[ag2_in[:]], outs=[ag2_out[:]],
            )

            # ---- transpose a rows -> aT [j, i] chunks (overlaps AG2;
            # transposes for adj emitted right after its post-chain).
            aT_sb = cpool.tile([128, 2, NCORES, 128], BF16, tag="aT_sb")
            cp_engines = [nc.vector, nc.scalar]
            for adj in range(2):
                for m in range(NCORES):
                    trp = pssmall.tile([128, 128], BF16, tag="ps",
                                       name=f"trp_{adj}_{m}")
                    nc.tensor.transpose(trp[:], a_sb[:, adj, m, :], ident[:])
                    eng = cp_engines[m % 2]
                    if eng is nc.vector:
                        nc.vector.tensor_copy(aT_sb[:, adj, m, :], trp[:])
                    else:
                        nc.scalar.activation(aT_sb[:, adj, m, :], trp[:],
                                             AF.Copy)

            # keep PE hot across the AG2 wait so the aggregation GEMMs run
            # at full clock (dep-free: these run back-to-back here)
            pe_warm(40, a_sb[:, 0, 0, :])

            # gather rotated G' chunks (64KB total), spread across queues
            for m in range(1, NCORES):
                xg_engines[(m - 1) % 3].dma_start(
                    G_full[:, m, :, :],
                    ag2_out[bass.ds(rot[m], 1)].rearrange("o a p f -> o p a f"))

            # ---- aggregate hT = sum G'^T(aT') + bgT; dinv_i already in aT,
            # dinv_j already in G', so both adjacencies share one PSUM.
            hT_ps = pssmall.tile([H, R], F32, tag="ps", name="hT_ps")
            nc.tensor.matmul(hT_ps[:], bg_sb[:], ones_row[:],
                             start=True, stop=False)
            for adj in range(2):
                for m in range(NCORES):
                    nc.tensor.matmul(hT_ps[:], G_full[:, m, adj, :],
                                     aT_sb[:, adj, m, :], start=False,
                                     stop=(adj == 1 and m == NCORES - 1))
            hT_bf = tpool.tile([H, R], BF16, tag="hT")
            nc.scalar.activation(hT_bf[:], hT_ps[:], AF.Relu)

            # out = h @ W_out + b_out (hT is already the lhsT layout)
            op = pssmall.tile([R, C], F32, tag="ps", name="op")
            nc.tensor.matmul(op[:], hT_bf[:], wout_sb[:], start=True, stop=False)
            nc.tensor.matmul(op[:], ones_row[:], bout_sb[:], start=False,
                             stop=True)
            out_sb = tpool.tile([R, C], F32, tag="osb")
            nc.vector.tensor_copy(out_sb[:], op[:])
            nc.sync.dma_start(out_dram[:], out_sb[:])

    nc.compile()
    return nc


_NC_CACHE = {}


def _get_nc():
    if "nc" not in _NC_CACHE:
        _NC_CACHE["nc"] = build_nc()
    return _NC_CACHE["nc"]


def _make_in_maps(inputs):
    waveforms = np.asarray(inputs["waveforms"], dtype=np.float32)
    spectrograms = np.asarray(inputs["spectrograms"], dtype=np.float32)
    W_time = np.asarray(inputs["W_time"], dtype=np.float32)
    W_freq = np.asarray(inputs["W_freq"], dtype=np.float32)
    W_gt = np.asarray(inputs["W_gt"], dtype=np.float32)
    W_gf = np.asarray(inputs["W_gf"], dtype=np.float32)
    W_out = np.asarray(inputs["W_out"], dtype=np.float32)
    b_time = np.asarray(inputs["b_time"], dtype=np.float32)
    b_freq = np.asarray(inputs["b_freq"], dtype=np.float32)
    b_g = np.asarray(inputs["b_g"], dtype=np.float32)
    b_out = np.asarray(inputs["b_out"], dtype=np.float32)

    T = waveforms.shape[1]

    def pmajor(arr_kN):
        # [KIN, 128] -> partition-major [128, KIN/128, 128]
        k = arr_kN.shape[0]
        return np.ascontiguousarray(
            arr_kN.reshape(k // 128, 128, -1).transpose(1, 0, 2))

    wt_pad = np.zeros((TPAD, D), dtype=bf)
    wt_pad[:T] = W_time.astype(bf)
    wf_b = W_freq.astype(bf)
    specs2 = spectrograms.reshape(N, FHW)

    common = dict(
        wt=pmajor(wt_pad),
        wf=pmajor(wf_b),
        wgt=np.ascontiguousarray(W_gt.astype(bf)),
        wgf=np.ascontiguousarray(W_gf.astype(bf)),
        wout=np.ascontiguousarray(W_out.astype(bf)),
        btime=np.ascontiguousarray(b_time.reshape(1, D).astype(bf)),
        bfreq=np.ascontiguousarray(b_freq.reshape(1, D).astype(bf)),
        bg=np.ascontiguousarray(b_g.reshape(1, H).astype(bf)),
        bout=np.ascontiguousarray(b_out.reshape(1, C).astype(bf)),
        ident=np.eye(128, dtype=bf),
    )
    in_maps = []
    for c in range(NCORES):
        rows = slice(c * R, (c + 1) * R)
        wT = np.zeros((TPAD, R), dtype=bf)
        wT[:T] = waveforms[rows].T.astype(bf)
        sT = specs2[rows].T.astype(bf)
        m = dict(common)
        m["wavesT"] = pmajor(wT)
        m["specsT"] = pmajor(sT)
        in_maps.append(m)
    return in_maps


def run(inputs, trace=False):
    nc = _get_nc()
    in_maps = _make_in_maps(inputs)
    res = run_bass_kernel_spmd(nc, in_maps, list(range(NCORES)), trace=trace)
    out = np.concatenate([res.results[c]["out"] for c in range(NCORES)], axis=0)
    return out.astype(np.float32), res


def kernel(**inputs):
    out, _ = run(inputs, trace=False)
    return out
